# revision 2
# baseline (speedup 1.0000x reference)
"""Two-layer GATv2 (heads=1, edge_dim=1) on 8 Trainium2 NeuronCores — v2.

Key changes vs baseline:
- Gather descriptors minimized: 3 overlapping int16 ranges (LP-balanced per
  block group), self-loops handled locally (no gather), group-level gathers.
- Edge phase has ZERO tensor-engine work: fold 1/|We_d| into the tables so the
  per-edge weight term is exactly +-w (sign sections by sign(We)); score uses a
  beta = att*|We| weighted reduce; aggregation reads the gathered xl~ directly.
- All per-block small ops batched at group level (kr-major slot layout keeps
  every broadcast/reduce a 3-dim AP).
"""

import numpy as np

N, E, D_IN, DH, DO = 50000, 800000, 128, 64, 32
C = 8
NL = N // C                # 6250
P = 128
NB = (NL + P - 1) // P     # 49
NLP = NB * P               # 6272
RB = [0, 8616, 17232]      # gather range bases (each 32768 rows wide)
RW = 32768
COLBUDGET = 84             # max slot columns per group (SBUF)


# ----------------------------------------------------------------------------
# host-side: weight folding
# ----------------------------------------------------------------------------

def _fold(Wl, bl, Wr, br, We, att, bias, in_perm=None, h_offset=False):
    We = np.asarray(We, np.float64)[:, 0]
    att = np.asarray(att, np.float64)
    pi = np.concatenate([np.nonzero(We >= 0)[0], np.nonzero(We < 0)[0]])
    p1 = int((We >= 0).sum())
    a = 1.0 / np.maximum(np.abs(We[pi]), 1e-8)      # v~ = a * v
    Wl = np.asarray(Wl, np.float64)[pi] * a[:, None]
    Wr = np.asarray(Wr, np.float64)[pi] * a[:, None]
    bl = np.asarray(bl, np.float64)[pi] * a
    br = np.asarray(br, np.float64)[pi] * a
    beta = att[pi] / a                               # score coef on m~
    if in_perm is not None:
        Wl = Wl[:, in_perm]
        Wr = Wr[:, in_perm]
    if h_offset:  # input arrives as h+1
        bl = bl - Wl.sum(1)
        br = br - Wr.sum(1)
    return dict(
        WlT=np.ascontiguousarray(Wl.T, np.float32),
        WrT=np.ascontiguousarray(Wr.T, np.float32),
        bl=bl.astype(np.float32)[:, None], br=br.astype(np.float32)[:, None],
        beta=beta.astype(np.float32),
        unscale=(1.0 / a).astype(np.float32),        # = |We[pi]|
        bias=np.asarray(bias, np.float64)[pi].astype(np.float32),
        pi=pi, p1=p1,
    )


# ----------------------------------------------------------------------------
# host-side: graph layout
# ----------------------------------------------------------------------------

def _group_lp(c0, c1, c2, c3, c4):
    """Min K0+K1+K2 for one group given per-node zone counts (Hall bounds)."""
    b0 = int(c0.max(initial=0)); b2 = int(c4.max(initial=0))
    b01 = int((c0 + c1).max(initial=0)); b12 = int((c3 + c4).max(initial=0))
    b02 = int((c0 + c4).max(initial=0))
    b012 = int((c0 + c1 + c2 + c3 + c4).max(initial=0))
    best = None
    for K0 in range(b0, b0 + 50):
        for K2 in range(b2, b2 + 50):
            if K0 + K2 < b02:
                continue
            K1 = max(b01 - K0, b12 - K2, b012 - K0 - K2, 0)
            s = K0 + K1 + K2
            if best is None or s < best[0]:
                best = (s, K0, K1, K2)
        if best and K0 >= b0 + 2 and best[0] <= K0:  # can't improve further
            break
    return best[1], best[2], best[3]


def _prep(x, edge_index, edge_weight):
    src = np.asarray(edge_index[0], np.int64)
    dst = np.asarray(edge_index[1], np.int64)
    w = np.asarray(edge_weight, np.float32)

    deg = np.bincount(dst, minlength=N)
    wsum = np.bincount(dst, weights=w.astype(np.float64), minlength=N)
    loop_w = (wsum / np.maximum(deg, 1)).astype(np.float32)

    order = np.argsort(-deg, kind="stable")
    ranks = np.arange(N)
    new_id = np.empty(N, np.int64)
    new_id[order] = (ranks % C) * NL + ranks // C
    inv = np.empty(N, np.int64)
    inv[new_id] = np.arange(N)               # old id of each new id
    rank_of = np.empty(N, np.int64)
    rank_of[(ranks % C) * NL + ranks // C] = ranks

    nsrc, ndst = new_id[src], new_id[dst]
    blk_of_rank = rank_of // (P * C)         # block 0..NB-1 (shared all cores)
    eblk = blk_of_rank[ndst]

    # ---- group blocks (consecutive) by degree proxy ----
    degs_sorted = deg[order]                 # descending by rank
    blocks_sizes = []
    b = 0
    while b < NB:
        maxd = int(degs_sorted[b * 1024]) + 6
        g = max(1, min(NB - b, COLBUDGET // maxd))
        blocks_sizes.append((b, b + g - 1))
        b += g

    # ---- AG chunks: coarse early, fine near the end (shrinks the AG tail)
    chunk_bounds = []                        # (lo, hi) in per-core local rows
    lo = 0
    tail_start = blocks_sizes[max(0, len(blocks_sizes) - 4)][0] * P
    for (b0, b1) in blocks_sizes:
        hi = min((b1 + 1) * P, NL)
        if hi <= tail_start:
            if hi - lo >= 3072:
                chunk_bounds.append((lo, hi))
                lo = hi
        else:
            if lo < b0 * P:
                chunk_bounds.append((lo, b0 * P))
                lo = b0 * P
            chunk_bounds.append((lo, hi))
            lo = hi
    if lo < NL:
        chunk_bounds.append((lo, NL))

    # ---- rid: table rows reordered (chunk, core, local) ----
    ag_rows = []                             # (lo, hi, off_row) per chunk
    rid_of_new = np.empty(N, np.int64)
    off = 0
    for (lo, hi) in chunk_bounds:
        rows = hi - lo
        for c in range(C):
            rid_of_new[c * NL + np.arange(lo, hi)] = \
                off + c * rows + np.arange(rows)
        ag_rows.append((lo, hi, off))
        off += C * rows
    assert off == N

    rsrc = rid_of_new[nsrc]
    # zones: 0:[0,8616) 1:[8616,17232) 2:[17232,32768) 3:[32768,41384) 4:rest
    zone = np.searchsorted([8616, 17232, 32768, 41384], rsrc, side="right")

    # per-node zone counts
    cz = [np.bincount(ndst[zone == z], minlength=N) for z in range(5)]
    nid_of_rank = np.empty(N, np.int64)
    nid_of_rank[rank_of[np.arange(N)]] = np.arange(N)  # rank -> new id
    czr = [c[nid_of_rank] for c in cz]       # zone counts by rank

    groups = []   # (b0, b1_inclusive, K0, K1, K2)
    for (b0, b1) in blocks_sizes:
        lo, hi = b0 * 1024, min((b1 + 1) * 1024, N)
        K0, K1, K2 = _group_lp(*[c[lo:hi] for c in czr])
        groups.append((b0, b1, K0, K1, K2))

    NG = len(groups)
    grp_of_blk = np.empty(NB, np.int64)
    for t, (b0, b1, *_) in enumerate(groups):
        grp_of_blk[b0:b1 + 1] = t

    # ---- per-edge range assignment (greedy, per node) ----
    egrp = grp_of_blk[eblk]
    Kg = np.array([[gq[2], gq[3], gq[4]] for gq in groups])  # [NG, 3]
    K0n = Kg[egrp, 0]; K2n = Kg[egrp, 2]                 # caps per edge's node
    # position of edge within its (node, zone) group
    okey = np.argsort(ndst * 8 + zone, kind="stable")
    pos = np.empty(E, np.int64)
    srt = (ndst * 8 + zone)[okey]
    starts = np.searchsorted(srt, srt)       # index of first equal element
    pos[okey] = np.arange(E) - starts
    c0n = cz[0][ndst]; c4n = cz[4][ndst]
    c1n = cz[1][ndst]; c3n = cz[3][ndst]
    rng = np.empty(E, np.int64)
    rng[zone == 0] = 0
    rng[zone == 4] = 2
    m1 = zone == 1
    rng[m1] = np.where(pos[m1] < (K0n - c0n)[m1], 0, 1)
    m3 = zone == 3
    rng[m3] = np.where(pos[m3] < (K2n - c4n)[m3], 2, 1)
    m2 = zone == 2
    rem0 = np.maximum(K0n - c0n - c1n, 0)
    rem2 = np.maximum(K2n - c4n - c3n, 0)
    rng[m2] = np.where(pos[m2] < rem0[m2], 0,
                       np.where(pos[m2] - rem0[m2] < rem2[m2], 2, 1))

    # slot index within (node, range)
    okey2 = np.argsort(ndst * 4 + rng, kind="stable")
    srt2 = (ndst * 4 + rng)[okey2]
    starts2 = np.searchsorted(srt2, srt2)
    kr = np.empty(E, np.int64)
    kr[okey2] = np.arange(E) - starts2
    for r in range(3):
        mm = rng == r
        assert (kr[mm] < Kg[egrp[mm], r]).all(), f"range {r} overflow"

    # ---- build slot tables in kr-major group layout ----
    # per group t: sections r=0,1,2 with cols_r = Kr*Gt, col = kr*Gt + g_local
    Gt = np.array([g[1] - g[0] + 1 for g in groups])
    sec_cols = (Kg * Gt[:, None])            # [NG, 3]
    grp_cols = sec_cols.sum(1)               # slot cols per group (no self)
    grp_off = np.concatenate([[0], np.cumsum(grp_cols)])
    TOTCOL = int(grp_off[-1])
    sec_off = np.zeros((NG, 3), np.int64)
    for t in range(NG):
        sec_off[t, 0] = grp_off[t]
        sec_off[t, 1] = grp_off[t] + sec_cols[t, 0]
        sec_off[t, 2] = grp_off[t] + sec_cols[t, 0] + sec_cols[t, 1]

    e_core = ndst // NL
    e_p = (ndst % NL) % P
    e_gl = eblk - np.array([groups[t][0] for t in egrp])  # g_local
    e_col = sec_off[egrp, rng] + kr * Gt[egrp] + e_gl

    w_slot = np.zeros((C, P, TOTCOL), np.float32)
    mask = np.zeros((C, P, TOTCOL), np.float32)
    w_slot[e_core, e_p, e_col] = w
    mask[e_core, e_p, e_col] = 1.0

    # ---- idx tiles: per (t, r) call, f = col_local*128 + p ----
    call_n = []          # num idxs per call, in (t, r) order
    for t in range(NG):
        for r in range(3):
            call_n.append(int(sec_cols[t, r]) * P)
    call_off = np.concatenate([[0], np.cumsum(call_n)])
    TOTIDX = int(call_off[-1])
    idx_cols = TOTIDX // 16
    idx_tab = np.zeros((C, 128, idx_cols), np.int16)
    call_id = egrp * 3 + rng
    col_local = kr * Gt[egrp] + e_gl
    f = call_off[call_id] + col_local * P + e_p
    idx_tab[e_core, f % 16, f // 16] = (rsrc - np.array(RB)[rng]).astype(np.int16)
    for rep in range(1, 8):
        idx_tab[:, 16 * rep:16 * rep + 16] = idx_tab[:, :16]

    # ---- misc per-core tables ----
    wloop = np.zeros((C, P, NB), np.float32)
    nid = np.arange(N)
    wloop[nid // NL, (nid % NL) % P, (nid % NL) // P] = loop_w[inv]

    x = np.asarray(x, np.float32)
    xT = np.zeros((C, D_IN, NLP), np.float32)
    perm = inv.reshape(C, NL)
    for c in range(C):
        xT[c, :, :NL] = x[perm[c]].T

    return dict(new_id=new_id, inv=inv, groups=groups, Gt=Gt, Kg=Kg,
                sec_cols=sec_cols, sec_off=sec_off, grp_off=grp_off,
                TOTCOL=TOTCOL, call_n=call_n, call_off=call_off,
                idx_tab=idx_tab, w_slot=w_slot, mask=mask, wloop=wloop, xT=xT,
                rid_of_new=rid_of_new, ag_rows=ag_rows,
                nsrc=nsrc, ndst=ndst, e_core=e_core, e_col=e_col, e_p=e_p)


# ----------------------------------------------------------------------------
# numpy simulator of the device program (for validation)
# ----------------------------------------------------------------------------

def _sim_edge_layer(g_core, plan, core, xl_nm, xr_nm, f, D):
    """g_core: [P, TOTCOL, D] gathered xl~ rows (already sliced to D).
    xl_nm/xr_nm: [P, NB, D]. Returns t5 [P, NB, D] (pre-activation+bias)."""
    groups, Gt, Kg = plan["groups"], plan["Gt"], plan["Kg"]
    sec_off, sec_cols = plan["sec_off"], plan["sec_cols"]
    w_slot = plan["w_slot"][core]
    mask = plan["mask"][core]
    wloop = plan["wloop"][core]
    beta, p1 = f["beta"][:D], f["p1"]
    out = np.zeros((P, NB, D), np.float32)
    for t, (b0, b1, K0, K1, K2) in enumerate(groups):
        G = b1 - b0 + 1
        e_parts = []
        aggs = []
        for r in range(3):
            Kr = Kg[t][r]
            o = sec_off[t][r]; nc_ = sec_cols[t][r]
            g = g_core[:, o:o + nc_, :]                  # [P, Kr*G, D]
            ws = w_slot[:, o:o + nc_]
            v = g.copy()
            v[:, :, :p1] += ws[:, :, None]
            v[:, :, p1:] -= ws[:, :, None]
            xr = xr_nm[:, b0:b1 + 1, :]                  # [P, G, D]
            v = v.reshape(P, Kr, G, D) + xr[:, None]
            m = np.where(v > 0, v, 0.2 * v)
            s = (m * beta).sum(-1)                       # [P, Kr, G]
            e = np.exp(s) * mask[:, o:o + nc_].reshape(P, Kr, G)
            e_parts.append(e)
            aggs.append((g.reshape(P, Kr, G, D), e))
        # self edge
        xl = xl_nm[:, b0:b1 + 1, :]
        xr = xr_nm[:, b0:b1 + 1, :]
        wl = wloop[:, b0:b1 + 1]
        vs = xl + xr
        vs = np.concatenate([vs[:, :, :p1] + wl[:, :, None],
                             vs[:, :, p1:] - wl[:, :, None]], axis=2)
        ms = np.where(vs > 0, vs, 0.2 * vs)
        es = np.exp((ms * beta).sum(-1))                 # [P, G]
        Z = es + sum(e.sum(1) for e in e_parts)          # [P, G]
        agg = es[:, :, None] * xl
        for gr, e in aggs:
            agg += (gr * e[:, :, :, None]).sum(1)
        out[:, b0:b1 + 1, :] = agg / Z[:, :, None]
    t5 = out * f["unscale"][:D] + f["bias"][:D]
    return t5


def _sim_forward(inputs, plan, f1, f2):
    """Full numpy simulation following the device dataflow."""
    x = np.asarray(inputs["x"], np.float32)
    inv = plan["inv"]
    out_all = np.zeros((C, NLP, DO), np.float32)
    # dense layer 1 (all cores)
    xl1 = np.zeros((C, P, NB, DH), np.float32)
    xr1 = np.zeros((C, P, NB, DH), np.float32)
    for c in range(C):
        xc = x[inv.reshape(C, NL)[c]]                    # [NL, 128]
        z = np.zeros((NLP, D_IN), np.float32)
        z[:NL] = xc
        xl = z @ f1["WlT"] + f1["bl"][:, 0]
        xr = z @ f1["WrT"] + f1["br"][:, 0]
        xl1[c] = xl.reshape(NB, P, DH).transpose(1, 0, 2)
        xr1[c] = xr.reshape(NB, P, DH).transpose(1, 0, 2)
    t1n = np.concatenate([
        xl1[c].transpose(1, 0, 2).reshape(NLP, DH)[:NL] for c in range(C)], 0)
    table1 = np.empty_like(t1n)
    table1[plan["rid_of_new"]] = t1n
    h_nm = np.zeros((C, P, NB, DH), np.float32)
    for c in range(C):
        gath = _gather_sim(table1, plan, c, DH)
        t5 = _sim_edge_layer(gath, plan, c, xl1[c], xr1[c], f1, DH)
        h_nm[c] = np.where(t5 > 0, t5, 0) + np.exp(np.minimum(t5, 0))  # elu+1
    # dense layer 2
    xl2 = np.zeros((C, P, NB, DO), np.float32)
    xr2 = np.zeros((C, P, NB, DO), np.float32)
    for c in range(C):
        h = h_nm[c].transpose(1, 0, 2).reshape(NLP, DH)
        xl = h @ f2["WlT"] + f2["bl"][:, 0]
        xr = h @ f2["WrT"] + f2["br"][:, 0]
        xl2[c] = xl.reshape(NB, P, DO).transpose(1, 0, 2)
        xr2[c] = xr.reshape(NB, P, DO).transpose(1, 0, 2)
    table2 = np.zeros((N, DH), np.float32)
    t2v = np.concatenate([
        xl2[c].transpose(1, 0, 2).reshape(NLP, DO)[:NL] for c in range(C)], 0)
    table2[plan["rid_of_new"], :DO] = t2v
    for c in range(C):
        gath = _gather_sim(table2, plan, c, DO)
        t5 = _sim_edge_layer(gath, plan, c, xl2[c], xr2[c], f2, DO)
        z = t5
        out_all[c] = (np.maximum(z, 0) + np.log1p(np.exp(-np.abs(z)))
                      + 1e-4).transpose(1, 0, 2).reshape(NLP, DO)
    full_new = np.concatenate([out_all[c][:NL] for c in range(C)], 0)
    full_old = full_new[plan["new_id"]]
    out = np.empty((N, DO), np.float32)
    out[:, f2["pi"]] = full_old
    return out


def _gather_sim(table, plan, core, D):
    """Simulate the idx-tile gathers: returns [P, TOTCOL, D]."""
    TOTCOL = plan["TOTCOL"]
    idx = plan["idx_tab"][core]
    call_n, call_off = plan["call_n"], plan["call_off"]
    g = np.zeros((P, TOTCOL, D), np.float32)
    NGc = len(call_n)
    col_base = 0
    for ci in range(NGc):
        n = call_n[ci]
        r = ci % 3
        base = RB[r]
        f = np.arange(call_off[ci], call_off[ci] + n)
        vals = idx[f % 16, f // 16].astype(np.int64) + base
        fl = f - call_off[ci]
        cols = col_base + fl // P
        ps = fl % P
        g[ps, cols, :] = table[vals, :D]
        col_base += n // P
    return g


# ----------------------------------------------------------------------------
# device program
# ----------------------------------------------------------------------------

SP = False  # dma_gather single_packet (True crashes at runtime)


def _build(plan, f1, f2):
    import concourse.bacc as bacc
    import concourse.bass as bass
    import concourse.mybir as mybir
    import concourse.tile as tile
    from concourse.library_config import mlp
    from concourse.masks import make_identity

    f32 = mybir.dt.float32
    bf16 = mybir.dt.bfloat16
    i16 = mybir.dt.int16
    Op = mybir.AluOpType
    Act = mybir.ActivationFunctionType

    groups = plan["groups"]
    Kg, Gt = plan["Kg"], plan["Gt"]
    call_n, call_off = plan["call_n"], plan["call_off"]
    grp_off = plan["grp_off"]
    TOTCOL = plan["TOTCOL"]
    NG = len(groups)
    idx_cols = plan["idx_tab"].shape[2]
    p1_1, p1_2 = f1["p1"], f2["p1"]

    nc = bacc.Bacc("TRN2", debug=False)

    def din(name, shape, dt=f32):
        return nc.dram_tensor(name, shape, dt, kind="ExternalInput")

    xT_d = din("xT", [D_IN, NLP], bf16)
    idx_d = din("idx_tab", [128, idx_cols], i16)
    w_d = din("w_slot", [P, TOTCOL])
    mask_d = din("mask", [P, TOTCOL])
    wloop_d = din("wloop", [P, NB])
    Wlr1_d = din("Wlr1", [D_IN, 2 * DH], bf16)
    bcat1_d = din("bcat1", [1, 2 * DH])
    Wlr2_d = din("Wlr2", [DH, 2 * DO], bf16)
    bcat2_d = din("bcat2", [1, 2 * DO])
    beta1_d, uns1_d, bias1_d = din("beta1", [1, DH]), din("uns1", [1, DH]), \
        din("bias1", [1, DH])
    beta2_d, uns2_d, bias2_d = din("beta2", [1, DO]), din("uns2", [1, DO]), \
        din("bias2", [1, DO])

    out_d = nc.dram_tensor("out", [NLP, DO], f32, kind="ExternalOutput")

    bounce1 = nc.dram_tensor("bounce1", [NL, DH], f32)
    table1 = nc.dram_tensor("table1", [N, DH], f32)
    bounce2 = nc.dram_tensor("bounce2", [NL, DH], f32)
    table2 = nc.dram_tensor("table2", [N, DH], f32)

    with tile.TileContext(nc) as tc:
      with tc.tile_pool(name="persist", bufs=1) as pp:
        ident = pp.tile([P, P], f32)
        make_identity(nc, ident[:])
        nc.gpsimd.load_library(mlp)
        c4_t = pp.tile([P, 1], f32)
        nc.vector.memset(c4_t[:], 1e-4)

        idx_t = pp.tile([128, idx_cols], i16)
        w_t = pp.tile([P, TOTCOL], f32)
        mask_t = pp.tile([P, TOTCOL], f32)
        wloop_t = pp.tile([P, NB], f32)
        xl1_nm = pp.tile([P, NB * DH], f32)
        xr1_nm = pp.tile([P, NB * DH], f32)
        h_t = pp.tile([P, NB * DH], f32)
        xl2_nm = pp.tile([P, NB * DO], f32)
        xr2_nm = pp.tile([P, NB * DO], f32)
        xr1_bf = pp.tile([P, NB * DH], bf16)
        xr2_bf = pp.tile([P, NB * DO], bf16)
        beta1_bf = pp.tile([P, DH], bf16)
        beta2_bf = pp.tile([P, DO], bf16)
        beta1_t = pp.tile([P, DH], f32)
        uns1_t = pp.tile([P, DH], f32)
        bias1_t = pp.tile([P, DH], f32)
        beta2_t = pp.tile([P, DO], f32)
        uns2_t = pp.tile([P, DO], f32)
        bias2_t = pp.tile([P, DO], f32)
        Wlr1_t = pp.tile([D_IN, 2 * DH], bf16)
        Wlr2_t = pp.tile([DH, 2 * DO], bf16)
        bcat1_t = pp.tile([P, 2 * DH], f32)
        bcat2_t = pp.tile([P, 2 * DO], f32)

        for t, d in [(idx_t, idx_d), (w_t, w_d), (mask_t, mask_d),
                     (wloop_t, wloop_d),
                     (Wlr1_t, Wlr1_d), (Wlr2_t, Wlr2_d)]:
            nc.sync.dma_start(t[:], d[:])
        for t, d, dd in [(beta1_t, beta1_d, DH), (uns1_t, uns1_d, DH),
                         (bias1_t, bias1_d, DH), (beta2_t, beta2_d, DO),
                         (uns2_t, uns2_d, DO), (bias2_t, bias2_d, DO),
                         (bcat1_t, bcat1_d, 2 * DH), (bcat2_t, bcat2_d,
                                                      2 * DO)]:
            nc.sync.dma_start(t[:], d[:].to_broadcast([P, dd]))
        nc.vector.tensor_copy(out=beta1_bf[:], in_=beta1_t[:])
        nc.vector.tensor_copy(out=beta2_bf[:], in_=beta2_t[:])

        def mid_bcast(ap2d, n):
            """[P, F] AP -> [P, (0,n), F]: broadcast a new middle dim."""
            return bass.AP(ap2d.tensor, ap2d.offset,
                           [ap2d.ap[0], [0, n], ap2d.ap[1]])

        def inner_bcast(ap2d, n):
            """[P, F] AP -> [P, F, (0,n)]: broadcast a new inner dim."""
            return bass.AP(ap2d.tensor, ap2d.offset, [*ap2d.ap, [0, n]])

        # ------------------ dense phase (either layer) -------------------
        # out[p, f] = sum_c lhsT[c, p] * Wcat[c, f]  -> node-major [xl | xr]
        def dense_block(j, DOUT, lhsT, Wcat, bcat, xl_nm, xr_nm, xr_bf,
                        bounce, pad, dps, dsb):
            ps = dps.tile([P, 2 * DOUT], f32, tag="mm", space="PSUM")
            nc.tensor.matmul(out=ps[:], lhsT=lhsT, rhs=Wcat,
                             start=True, stop=True)
            wb = dsb.tile([P, 2 * DOUT], f32, tag="wb")
            nc.vector.tensor_tensor(out=wb[:], in0=ps[:], in1=bcat,
                                    op=Op.add)
            nc.scalar.activation(out=xl_nm[:, j * DOUT:(j + 1) * DOUT],
                                 in_=wb[:, 0:DOUT], func=Act.Identity,
                                 bias=0.0)
            nc.scalar.activation(out=xr_nm[:, j * DOUT:(j + 1) * DOUT],
                                 in_=wb[:, DOUT:2 * DOUT],
                                 func=Act.Identity, bias=0.0)
            nc.vector.tensor_copy(out=xr_bf[:, j * DOUT:(j + 1) * DOUT],
                                  in_=wb[:, DOUT:2 * DOUT])
            st = dsb.tile([P, DH], f32, tag="st")
            if pad:
                nc.vector.memset(st[:], 0.0)
            nc.vector.tensor_copy(out=st[:, :DOUT], in_=wb[:, 0:DOUT])
            lo, hi = j * P, min((j + 1) * P, NL)
            if hi > lo:
                nc.sync.dma_start(out=bounce[lo:hi, :], in_=st[:hi - lo, :])

        def dense(DOUT, rhsT_ap, Wcat, bcat, xl_nm, xr_nm, xr_bf,
                  bounce, pad):
            with (
                tc.tile_pool(name="dps", bufs=2, space="PSUM") as dps,
                tc.tile_pool(name="dsb", bufs=3) as dsb,
            ):
                for j in range(NB):
                    dense_block(j, DOUT, rhsT_ap[:, j * P:(j + 1) * P], Wcat,
                                bcat, xl_nm, xr_nm, xr_bf, bounce,
                                pad, dps, dsb)

        # ------------------ edge phase (either layer) --------------------
        def edge(layer):
            if layer == 1:
                D, p1, table = DH, p1_1, table1
                xl_nm, xr_nm, xr_bfl = xl1_nm, xr1_nm, xr1_bf
                beta_r, beta_b, uns_r, bias_r = beta1_t, beta1_bf, uns1_t, \
                    bias1_t
            else:
                D, p1, table = DO, p1_2, table2
                xl_nm, xr_nm, xr_bfl = xl2_nm, xr2_nm, xr2_bf
                beta_r, beta_b, uns_r, bias_r = beta2_t, beta2_bf, uns2_t, \
                    bias2_t

            with (
                tc.tile_pool(name=f"gp{layer}", bufs=3) as gp,
                tc.tile_pool(name=f"wk{layer}", bufs=1) as wk,
                tc.tile_pool(name=f"sm{layer}", bufs=1) as sm,
                tc.tile_pool(name=f"d2p{layer}", bufs=2, space="PSUM") as d2ps,
                tc.tile_pool(name=f"d2s{layer}", bufs=3) as d2sb,
            ):
              for t in range(NG):
                b0, b1, K0, K1, K2 = groups[t]
                G = b1 - b0 + 1
                Ks = (K0, K1, K2)
                cols = (K0 + K1 + K2) * G          # slot cols (no self)
                go = int(grp_off[t])               # global col offset
                secs = [0, K0 * G, (K0 + K1) * G]  # local sec offsets

                g_t = gp.tile([P, cols, DH], f32, tag="g")
                for r in range(3):
                    n = int(call_n[t * 3 + r])
                    if n == 0:
                        continue
                    o16 = int(call_off[t * 3 + r]) // 16
                    nc.gpsimd.dma_gather(
                        g_t[:, secs[r]:secs[r] + n // P, :],
                        table[RB[r]:RB[r] + RW, :],
                        idx_t[:, o16:o16 + n // 16], n, n, DH,
                        single_packet=SP)

                xl_g = xl_nm[:, b0 * D:(b1 + 1) * D]   # [P, G*D]
                xr_g = xr_nm[:, b0 * D:(b1 + 1) * D]
                xr_bfg = xr_bfl[:, b0 * D:(b1 + 1) * D]
                # scores tile: slot sections + self section
                s_t = sm.tile([P, cols + G], f32, tag="s")
                v_t = wk.tile([P, cols, D], bf16, tag="v")
                m_t = wk.tile([P, cols, D], bf16, tag="m")
                for r in range(3):
                    if Ks[r] == 0:
                        continue
                    sec = slice(secs[r], secs[r] + Ks[r] * G)
                    wsl = w_t[:, go + secs[r]:go + secs[r] + Ks[r] * G]
                    # v = g +- w  (sign sections by sign(We))
                    if p1 > 0:
                        nc.vector.tensor_tensor(
                            out=v_t[:, sec, 0:p1], in0=g_t[:, sec, 0:p1],
                            in1=inner_bcast(wsl, p1), op=Op.add)
                    if p1 < D:
                        nc.vector.tensor_tensor(
                            out=v_t[:, sec, p1:D], in0=g_t[:, sec, p1:D],
                            in1=inner_bcast(wsl, D - p1), op=Op.subtract)
                    # m_t <- v + xr (broadcast over kr)
                    nc.vector.tensor_tensor(
                        out=m_t[:, sec, :].rearrange(
                            "p (k g) d -> p k (g d)", g=G),
                        in0=v_t[:, sec, :].rearrange(
                            "p (k g) d -> p k (g d)", g=G),
                        in1=mid_bcast(xr_bfg, Ks[r]), op=Op.add)
                    # v_t <- lrelu(m_t)
                    nc.scalar.activation(out=v_t[:, sec, :],
                                         in_=m_t[:, sec, :],
                                         func=Act.Prelu, alpha=0.2)
                    # m_t <- v_t * beta ; s = sum_d
                    nc.vector.tensor_tensor(
                        out=m_t[:, sec, :], in0=v_t[:, sec, :],
                        in1=mid_bcast(beta_b[:, 0:D], Ks[r] * G), op=Op.mult)
                    nc.vector.tensor_reduce(
                        out=s_t[:, sec], in_=m_t[:, sec, :],
                        axis=mybir.AxisListType.X, op=Op.add)
                # self edge: v = xl + xr +- wloop
                vs_t = sm.tile([P, G, D], f32, tag="vs")
                ms_t = sm.tile([P, G, D], f32, tag="ms")
                wl_g = wloop_t[:, b0:b1 + 1]
                xl3 = xl_g.rearrange("p (g d) -> p g d", d=D)
                xr3 = xr_g.rearrange("p (g d) -> p g d", d=D)
                if p1 > 0:
                    nc.vector.tensor_tensor(
                        out=vs_t[:, :, 0:p1], in0=xl3[:, :, 0:p1],
                        in1=inner_bcast(wl_g, p1), op=Op.add)
                if p1 < D:
                    nc.vector.tensor_tensor(
                        out=vs_t[:, :, p1:D], in0=xl3[:, :, p1:D],
                        in1=inner_bcast(wl_g, D - p1), op=Op.subtract)
                nc.vector.tensor_tensor(out=ms_t[:], in0=vs_t[:], in1=xr3,
                                        op=Op.add)
                nc.scalar.activation(out=vs_t[:], in_=ms_t[:],
                                     func=Act.Prelu, alpha=0.2)
                nc.vector.tensor_tensor(
                    out=ms_t[:], in0=vs_t[:],
                    in1=mid_bcast(beta_r[:, 0:D], G), op=Op.mult)
                nc.vector.tensor_reduce(
                    out=s_t[:, cols:cols + G], in_=ms_t[:],
                    axis=mybir.AxisListType.X, op=Op.add)

                # e = exp(s); mask slots; Z; alpha
                e_t = sm.tile([P, cols + G], f32, tag="e")
                nc.scalar.activation(out=e_t[:], in_=s_t[:], func=Act.Exp)
                e2_t = sm.tile([P, cols + G], f32, tag="e2")
                nc.vector.tensor_tensor(out=e2_t[:, 0:cols],
                                        in0=e_t[:, 0:cols],
                                        in1=mask_t[:, go:go + cols],
                                        op=Op.mult)
                Z_t = sm.tile([P, G], f32, tag="Z")
                Zp_t = sm.tile([P, G], f32, tag="Zp")
                nc.scalar.activation(out=Z_t[:], in_=e_t[:, cols:cols + G],
                                     func=Act.Identity, bias=0.0)
                for r in range(3):
                    if Ks[r] == 0:
                        continue
                    er = e2_t[:, secs[r]:secs[r] + Ks[r] * G]
                    nc.vector.tensor_reduce(
                        out=Zp_t[:],
                        in_=er.rearrange("p (k g) -> p g k", g=G),
                        axis=mybir.AxisListType.X, op=Op.add)
                    nc.vector.tensor_tensor(out=Z_t[:], in0=Z_t[:],
                                            in1=Zp_t[:], op=Op.add)
                iZ_t = sm.tile([P, G], f32, tag="iZ")
                nc.vector.reciprocal(out=iZ_t[:], in_=Z_t[:])
                al_t = sm.tile([P, cols + G], f32, tag="al")
                for r in range(3):
                    if Ks[r] == 0:
                        continue
                    sec = slice(secs[r], secs[r] + Ks[r] * G)
                    nc.vector.tensor_tensor(
                        out=al_t[:, sec], in0=e2_t[:, sec],
                        in1=mid_bcast(iZ_t[:], Ks[r]), op=Op.mult)
                nc.vector.tensor_tensor(out=al_t[:, cols:cols + G],
                                        in0=e_t[:, cols:cols + G],
                                        in1=iZ_t[:], op=Op.mult)

                # agg = sum_k alpha * g  (+ self)
                agg_t = sm.tile([P, G * D], f32, tag="agg")
                agp_t = sm.tile([P, G * D], f32, tag="agp")
                nc.vector.tensor_tensor(
                    out=agg_t[:].rearrange("p (g d) -> p g d", d=D),
                    in0=xl3,
                    in1=inner_bcast(al_t[:, cols:cols + G], D), op=Op.mult)
                for r in range(3):
                    if Ks[r] == 0:
                        continue
                    sec = slice(secs[r], secs[r] + Ks[r] * G)
                    # p = alpha * g -> reuse m_t
                    nc.vector.tensor_tensor(
                        out=m_t[:, sec, 0:D], in0=g_t[:, sec, 0:D],
                        in1=inner_bcast(al_t[:, sec], D), op=Op.mult)
                    pr = m_t[:, sec, 0:D]
                    nc.vector.tensor_reduce(
                        out=agp_t[:],
                        in_=pr.rearrange("p (k g) d -> p (g d) k", g=G),
                        axis=mybir.AxisListType.X, op=Op.add)
                    nc.vector.tensor_tensor(out=agg_t[:], in0=agg_t[:],
                                            in1=agp_t[:], op=Op.add)

                # t5 = agg * unscale + bias
                t5_t = sm.tile([P, G * D], f32, tag="t5")
                nc.vector.tensor_tensor(
                    out=t5_t[:].rearrange("p (g d) -> p g d", d=D),
                    in0=agg_t[:].rearrange("p (g d) -> p g d", d=D),
                    in1=mid_bcast(uns_r[:, 0:D], G), op=Op.mult)
                nc.vector.tensor_tensor(
                    out=t5_t[:].rearrange("p (g d) -> p g d", d=D),
                    in0=t5_t[:].rearrange("p (g d) -> p g d", d=D),
                    in1=mid_bcast(bias_r[:, 0:D], G), op=Op.add)

                if layer == 1:
                    # h = elu(t5)+1 = relu(t5) + exp(min(t5,0))
                    r_t = sm.tile([P, G * D], f32, tag="r")
                    nc.scalar.activation(out=r_t[:], in_=t5_t[:],
                                         func=Act.Relu)
                    u_t = sm.tile([P, G * D], f32, tag="u")
                    nc.vector.tensor_tensor(out=u_t[:], in0=t5_t[:],
                                            in1=r_t[:], op=Op.subtract)
                    u2_t = sm.tile([P, G * D], f32, tag="u2")
                    nc.scalar.activation(out=u2_t[:], in_=u_t[:],
                                         func=Act.Exp)
                    nc.vector.tensor_tensor(
                        out=h_t[:, b0 * DH:(b1 + 1) * DH], in0=r_t[:],
                        in1=u2_t[:], op=Op.add)
                    # interleaved layer-2 dense for this group's blocks
                    for j in range(b0, b1 + 1):
                        pst = d2ps.tile([DH, P], f32, tag="ht", space="PSUM")
                        nc.tensor.transpose(
                            out=pst[:], in_=h_t[:, j * DH:(j + 1) * DH],
                            identity=ident[:])
                        hTb = d2sb.tile([DH, P], bf16, tag="hTb")
                        nc.scalar.activation(out=hTb[:], in_=pst[:],
                                             func=Act.Identity, bias=0.0)
                        dense_block(j, DO, hTb[:], Wlr2_t[:], bcat2_t[:],
                                    xl2_nm, xr2_nm,
                                    xr2_bf, bounce2, True, d2ps, d2sb)
                else:
                    # softplus(z)+1e-4 = relu(z) + ln(1.0001 + 1.0001*e^-|z|)
                    ab_t = sm.tile([P, G * D], f32, tag="ab")
                    nc.scalar.activation(out=ab_t[:], in_=t5_t[:],
                                         func=Act.Abs)
                    ex_t = sm.tile([P, G * D], f32, tag="ex")
                    nc.scalar.activation(out=ex_t[:], in_=ab_t[:],
                                         func=Act.Exp, scale=-1.0)
                    ln_t = sm.tile([P, G * D], f32, tag="ln")
                    nc.scalar.activation(out=ln_t[:], in_=ex_t[:],
                                         func=Act.Ln, bias=1.0)
                    sp_t = sm.tile([P, G * D], f32, tag="sp")
                    nc.vector.scalar_tensor_tensor(
                        out=sp_t[:], in0=t5_t[:], scalar=0.0, in1=ln_t[:],
                        op0=Op.max, op1=Op.add)
                    o_t = sm.tile([P, G * D], f32, tag="o")
                    nc.vector.tensor_tensor(
                        out=o_t[:], in0=sp_t[:],
                        in1=bass.AP(c4_t[:].tensor, c4_t[:].offset,
                                    [c4_t[:].ap[0], [0, G * D]]),
                        op=Op.add)
                    for j in range(b0, b1 + 1):
                        nc.sync.dma_start(
                            out=out_d[j * P:(j + 1) * P, :],
                            in_=o_t[:, (j - b0) * DO:(j - b0 + 1) * DO])

        # ---------------------------- schedule ---------------------------
        with tc.tile_pool(name="xt", bufs=1) as xtp:
            xT_s = xtp.tile([D_IN, NLP], bf16)
            nc.sync.dma_start(xT_s[:], xT_d[:])
            dense(DH, xT_s[:], Wlr1_t[:], bcat1_t[:],
                  xl1_nm, xr1_nm, xr1_bf, bounce1, pad=False)

        def ag_split(bounce, table):
            for (lo, hi, off) in plan["ag_rows"]:
                nc.gpsimd.collective_compute(
                    "AllGather", Op.bypass,
                    replica_groups=[list(range(C))],
                    ins=[bounce[lo:hi, :]],
                    outs=[table[off:off + C * (hi - lo), :]])

        ag_split(bounce1, table1)
        edge(1)
        ag_split(bounce2, table2)
        edge(2)

    nc.compile()
    return nc


# ----------------------------------------------------------------------------
# entry point
# ----------------------------------------------------------------------------

def _make_in_maps(inputs):
    f1 = _fold(inputs["Wl1"], inputs["bl1"], inputs["Wr1"], inputs["br1"],
               inputs["We1"], inputs["att1"], inputs["bias1"])
    f2 = _fold(inputs["Wl2"], inputs["bl2"], inputs["Wr2"], inputs["br2"],
               inputs["We2"], inputs["att2"], inputs["bias2"],
               in_perm=f1["pi"], h_offset=True)
    plan = _prep(inputs["x"], inputs["edge_index"], inputs["edge_weight"])
    import ml_dtypes
    bf = ml_dtypes.bfloat16
    shared = dict(
        Wlr1=np.concatenate([f1["WlT"], f1["WrT"]], 1).astype(bf),
        bcat1=np.concatenate([f1["bl"][:, 0], f1["br"][:, 0]])[None, :],
        Wlr2=np.concatenate([f2["WlT"], f2["WrT"]], 1).astype(bf),
        bcat2=np.concatenate([f2["bl"][:, 0], f2["br"][:, 0]])[None, :],
        beta1=f1["beta"][None, :], uns1=f1["unscale"][None, :],
        bias1=f1["bias"][None, :],
        beta2=f2["beta"][None, :], uns2=f2["unscale"][None, :],
        bias2=f2["bias"][None, :],
    )
    in_maps = []
    for c in range(C):
        m = dict(shared)
        m.update(xT=plan["xT"][c].astype(bf), idx_tab=plan["idx_tab"][c],
                 w_slot=plan["w_slot"][c], mask=plan["mask"][c],
                 wloop=plan["wloop"][c])
        in_maps.append(m)
    return plan, in_maps, f1, f2


def kernel(**inputs):
    from concourse.bass_utils import run_bass_kernel_spmd

    plan, in_maps, f1, f2 = _make_in_maps(inputs)
    nc = _build(plan, f1, f2)
    res = run_bass_kernel_spmd(nc, in_maps, list(range(C)))

    full_new = np.concatenate([res.results[c]["out"][:NL] for c in range(C)],
                              0)
    full_old = full_new[plan["new_id"]]
    out = np.empty((N, DO), np.float32)
    out[:, f2["pi"]] = full_old
    return out.astype(np.float32)


def validate():
    import reference
    import jax
    cpu = jax.local_devices(backend="cpu")[0]
    with jax.default_device(cpu):
        inputs = {k: np.asarray(v) for k, v in reference.setup_inputs().items()}
        expected = np.asarray(jax.jit(reference.reference)(**inputs))
    f1 = _fold(inputs["Wl1"], inputs["bl1"], inputs["Wr1"], inputs["br1"],
               inputs["We1"], inputs["att1"], inputs["bias1"])
    f2 = _fold(inputs["Wl2"], inputs["bl2"], inputs["Wr2"], inputs["br2"],
               inputs["We2"], inputs["att2"], inputs["bias2"],
               in_perm=f1["pi"], h_offset=True)
    plan = _prep(inputs["x"], inputs["edge_index"], inputs["edge_weight"])
    print("groups:", plan["groups"])
    print("TOTCOL:", plan["TOTCOL"], "descs/layer:",
          sum(plan["call_n"]), "ncalls:", len(plan["call_n"]))
    actual = _sim_forward(inputs, plan, f1, f2)
    err = np.abs(actual - expected)
    rel = err.max() / np.abs(expected).max()
    print(f"sim rel err: {rel:.3e}")


if __name__ == "__main__":
    validate()
